# revision 36
# baseline (speedup 1.0000x reference)
"""Multi-head attention (RoPE, causal) Trainium2 Bass kernel, 8-way sharded.

Sharding: core c handles batch b = c//4 and heads 4*(c%4)..4*(c%4)+3
(B*H = 32 head-rows -> 4 per core).  QKV/out projections are
Megatron-sliced per core; per-core partial outputs (row-parallel Wo)
are summed on the host.

Problem constants (hardcoded per contract):
  B=2, S=2048, D=1024, H=16, DK=64
"""

import math

import ml_dtypes
import numpy as np

import concourse.bass as bass
import concourse.mybir as mybir
import concourse.tile as tile
from concourse import bacc
from concourse.bass_utils import run_bass_kernel_spmd

B, S, D, H, DK = 2, 2048, 1024, 16, 64
E = 256            # head dims per core (4 heads x 64)
CH = 512           # sequence chunk (matmul free dim)
NCH = S // CH      # 4
NST = S // 128     # 16 s-tiles
BF16 = mybir.dt.bfloat16
F32 = mybir.dt.float32


def _np_reference_fallback(q, k, v, mask, Wq, bq, Wk, bk, Wv, bv, Wo, bo):
    """Pure-numpy reference path (only used for inputs outside the
    contract: non-causal mask or nonzero qkv biases)."""
    qh = (q @ Wq.T + bq).reshape(B, S, H, DK)
    kh = (k @ Wk.T + bk).reshape(B, S, H, DK)
    vh = (v @ Wv.T + bv).reshape(B, S, H, DK)
    inv_freq = 1.0 / (10000.0 ** (np.arange(0, DK, 2, dtype=np.float32) / DK))
    pos = np.arange(S, dtype=np.float32)
    fr = pos[:, None] * inv_freq[None, :]
    cos, sin = np.cos(fr)[:, None, :], np.sin(fr)[:, None, :]

    def rope(x):
        t = DK // 2
        x1, x2 = x[..., :t], x[..., t:]
        return np.concatenate([x1 * cos - x2 * sin, x1 * sin + x2 * cos], -1)

    qh, kh = rope(qh), rope(kh)
    sc = np.einsum('bqhd,bkhd->bhqk', qh, kh) / math.sqrt(DK)
    sc = np.where(mask == 0, np.float32(-10000.0), sc)
    sc = sc - sc.max(-1, keepdims=True)
    e = np.exp(sc)
    attn = e / e.sum(-1, keepdims=True)
    out = np.einsum('bhqk,bkhd->bqhd', attn, vh).reshape(B, S, D)
    return (out @ Wo.T + bo).astype(np.float32)


def _build_program():
    nc = bacc.Bacc(None, target_bir_lowering=False)

    dp = nc.declare_dram_parameter
    xq = dp("xq", [D, S], BF16, isOutput=False)   # q[b].T
    xk = dp("xk", [D, S], BF16, isOutput=False)
    xv = dp("xv", [D, S], BF16, isOutput=False)
    wq = dp("wq", [D, E], BF16, isOutput=False)   # Wq_c.T
    wk = dp("wk", [D, E], BF16, isOutput=False)
    wv = dp("wv", [D, E], BF16, isOutput=False)
    wo = dp("wo", [E, D], BF16, isOutput=False)   # Wo_c.T rows
    ct = dp("ct", [E, S], BF16, isOutput=False)   # cos table (1/sqrt8 folded)
    st = dp("st", [E, S], BF16, isOutput=False)   # signed sin table
    rt = dp("rt", [128, 128], BF16, isOutput=False)  # half-swap permutation
    tri = dp("tri", [128, 896], BF16, isOutput=False)  # wide causal 0/1 ramp
    out = dp("out", [S, D], BF16, isOutput=True)
    den_d = nc.dram_tensor("den_d", [16, CH], F32)   # denominator bounce rows
    rec_d = nc.dram_tensor("rec_d", [16, CH], F32)   # reciprocal bounce rows

    with tile.TileContext(nc) as tc:
        with (
            tc.tile_pool(name="const", bufs=1) as const,
            tc.tile_pool(name="persist", bufs=1) as persist,
            tc.tile_pool(name="xt", bufs=6) as xtp,
            tc.tile_pool(name="xts", bufs=16) as xtsp,
            tc.tile_pool(name="raw", bufs=4) as rawp,
            tc.tile_pool(name="ropetmp", bufs=4) as rtp,
            tc.tile_pool(name="pblk", bufs=4) as pblk,
            tc.tile_pool(name="ppart", bufs=4) as ppart,
            tc.tile_pool(name="norm", bufs=6) as normp,
            tc.tile_pool(name="obuf", bufs=4) as obufp,
            tc.tile_pool(name="projps", bufs=2, space="PSUM") as projps,
        ):
            # ---- constants to SBUF ----
            # warm-up tile needs no DMA: memset so PE can start immediately
            warm_t = const.tile([128, 128], BF16, tag="warm")
            nc.vector.memset(warm_t[:], 0.02)
            rt_t = const.tile([128, 128], BF16, tag="rt")
            wk_t = const.tile([128, 8, E], BF16, tag="wk")
            wq_t = const.tile([128, 8, E], BF16, tag="wq")
            ct_t = const.tile([128, 2, S], BF16, tag="ct")
            st_t = const.tile([128, 2, S], BF16, tag="st")
            wv_t = const.tile([128, 8, E], BF16, tag="wv")
            tri_t = const.tile([128, 896], BF16, tag="tri")
            wo_t = const.tile([128, 2, D], BF16, tag="wo")
            wo_h = const.tile([64, D], BF16, tag="wo_h")  # pair1-hi rows at base 0
            ones_t = const.tile([1, 64], F32, tag="ones")
            nc.vector.memset(ones_t[:], 1.0)

            # ---- persistent intermediates ----
            qT = persist.tile([128, 2, S], BF16, tag="qT")   # partitions: e%128, dim1: e//128
            kT = persist.tile([128, 2, S], BF16, tag="kT")
            aT = persist.tile([128, 2, S], BF16, tag="aT")
            vext = persist.tile([128, NST, 4, 65], BF16, tag="vext")
            nc.vector.memset(vext[:, :, :, 64:65], 1.0)

            def load_x_chunk(x_dram, c, dma_eng, split=False):
                xsrc = x_dram[:].rearrange("(kt p) s -> p kt s", p=128)
                if split:
                    # independent per-kt tiles: the first proj matmul only
                    # waits on its own kt slice, not the whole chunk
                    xts = []
                    for kt in range(8):
                        xk1 = xtsp.tile([128, CH], BF16, tag="xts")
                        dma_eng.dma_start(out=xk1[:], in_=xsrc[:, kt, c * CH:(c + 1) * CH])
                        xts.append(xk1)
                    return xts
                xt = xtp.tile([128, 8, CH], BF16, tag="xt")
                dma_eng.dma_start(out=xt[:], in_=xsrc[:, :, c * CH:(c + 1) * CH])
                return xt

            def proj_rope_chunk(xt, w_t, dest, c):
                """Project chunk c of q/k into [e, s] layout + rope.

                Both m-blocks' projection matmuls are issued before either
                rope rotation so the PE never stalls on the DVE copy."""
                raws = []
                for m in range(2):
                    ps = projps.tile([128, CH], F32, tag="ps")
                    for kt in range(8):
                        nc.tensor.matmul(
                            ps[:], lhsT=w_t[:, kt, m * 128:(m + 1) * 128],
                            rhs=(xt[kt][:] if isinstance(xt, list)
                                 else xt[:, kt, :]),
                            start=(kt == 0), stop=(kt == 7),
                        )
                    raw = rawp.tile([128, CH], BF16, tag="raw")
                    nc.vector.tensor_copy(raw[:], ps[:])
                    raws.append(raw)
                for m in range(2):
                    raw = raws[m]
                    rps = projps.tile([128, CH], F32, tag="ps")
                    nc.tensor.matmul(rps[:], lhsT=rt_t[:], rhs=raw[:], start=True, stop=True)
                    t1 = rtp.tile([128, CH], BF16, tag="rtmp")
                    nc.vector.tensor_mul(t1[:], rps[:], st_t[:, m, c * CH:(c + 1) * CH])
                    t2 = rtp.tile([128, CH], BF16, tag="rtmp")
                    nc.vector.tensor_mul(t2[:], raw[:], ct_t[:, m, c * CH:(c + 1) * CH])
                    nc.vector.tensor_add(dest[:, m, c * CH:(c + 1) * CH], t1[:], t2[:])

            def vproj_stile(xt_v, stl):
                """Project s-tile stl of v into vext [s, (h, e)] layout."""
                ps = projps.tile([128, E], F32, tag="ps")
                for kt in range(8):
                    lhsT = (xt_v[kt][:, (stl % 4) * 128:(stl % 4) * 128 + 128]
                            if isinstance(xt_v, list)
                            else xt_v[:, kt, (stl % 4) * 128:(stl % 4) * 128 + 128])
                    nc.tensor.matmul(
                        ps[:], lhsT=lhsT,
                        rhs=wv_t[:, kt, :], start=(kt == 0), stop=(kt == 7),
                    )
                nc.vector.tensor_copy(
                    vext[:, stl, :, 0:64],
                    ps[:].rearrange("p (h e) -> p h e", h=4),
                )

            def attention_chunk(c, spair, opsum, filler=None, post_pair=None):
                nj = 4 * c + 4
                oc_tiles = []
                for pair in range(2):
                    if filler is not None:
                        filler(pair)
                    o_lo = opsum.tile([65, CH], F32, tag="o")
                    o_hi = opsum.tile([65, CH], F32, tag="o")
                    h_lo, h_hi = 2 * pair, 2 * pair + 1
                    # -- all sk-tile pairs; diagonal tiles masked post-exp --
                    for jj in range(0, nj, 2):
                        # leading columns that are fully masked for both tiles
                        gmin = max(0, (jj - 4 * c) * 128)
                        for half, ob, pt in ((0, o_lo, 0), (64, o_hi, 1)):
                            sp = spair.tile([128, 2 * CH], F32, tag="sp")
                            for dj in range(2):
                                j = jj + dj
                                g0 = dj * CH + (gmin if dj == 0 else 0)
                                nc.tensor.matmul(
                                    sp[:, g0:(dj + 1) * CH],
                                    lhsT=kT[half:half + 64, pair, j * 128:(j + 1) * 128],
                                    rhs=qT[half:half + 64, pair,
                                           c * CH + (g0 - dj * CH):(c + 1) * CH],
                                    start=True, stop=True,
                                )
                            p = pblk.tile([128, 2 * CH], BF16, tag="p")
                            nc.scalar.activation(
                                p[:, gmin:], sp[:, gmin:],
                                mybir.ActivationFunctionType.Exp)
                            for dj in range(2):
                                j = jj + dj
                                # leading fully-masked cols of diagonal tiles
                                # are skipped by the trimmed AV matmul, so the
                                # mask multiply only needs the boundary block
                                g = (j - 4 * c) * 128 if j >= 4 * c else 0
                                if j >= 4 * c:
                                    nc.gpsimd.tensor_mul(
                                        p[:, dj * CH + g:dj * CH + g + 128],
                                        p[:, dj * CH + g:dj * CH + g + 128],
                                        tri_t[:, 384:512],
                                    )
                                nc.tensor.matmul(
                                    ob[:, g:CH], lhsT=vext[:, j, 2 * pair + pt, :],
                                    rhs=p[:, dj * CH + g:(dj + 1) * CH],
                                    start=(j == 0), stop=(j == nj - 1),
                                    skip_group_check=True,
                                )
                    # -- evict O to SBUF (frees PSUM); stash oc tiles.  The
                    # final pair evicts on the scalar engine (idle once exps
                    # are done) so the DVE can go straight to the reciprocals.
                    tail_pair = (c == NCH - 1 and pair == 1)
                    evict = nc.scalar.copy if tail_pair else (
                        lambda out, in_: nc.vector.tensor_copy(out, in_))
                    oc_lo = normp.tile([65, CH], F32, tag="oc")
                    evict(oc_lo[:], o_lo[:])
                    oc_hi = normp.tile([65, CH], F32, tag="oc")
                    evict(oc_hi[:], o_hi[:])
                    oc_tiles.append((pair, oc_lo, oc_hi))
                    if post_pair is not None:
                        post_pair(pair, oc_lo, oc_hi)
                return oc_tiles

            def normalize_pair_bounce(c, pair, oc_lo, oc_hi):
                # DRAM-bounce reciprocal for one pair (2 denominator rows at
                # once); all DMA/DVE, no PE instructions.
                r0 = 4 * c + 2 * pair
                nc.gpsimd.dma_start(
                    out=den_d[:][r0:r0 + 1, :], in_=oc_lo[64:65, :])
                nc.gpsimd.dma_start(
                    out=den_d[:][r0 + 1:r0 + 2, :], in_=oc_hi[64:65, :])
                sc16 = normp.tile([128, 2, CH // 128], F32, tag="sc16")
                nc.gpsimd.dma_start(
                    out=sc16[:],
                    in_=den_d[:].rearrange("r (p j) -> p r j", p=128)[:, r0:r0 + 2, :])
                rc16 = normp.tile([128, 2, CH // 128], F32, tag="rc16")
                nc.vector.reciprocal(rc16[:], sc16[:])
                nc.gpsimd.dma_start(
                    out=rec_d[:].rearrange("r (p j) -> p r j", p=128)[:, r0:r0 + 2, :],
                    in_=rc16[:])
                rsrc = rec_d[:]
                for half, oc in ((0, oc_lo), (64, oc_hi)):
                    ridx = r0 + (half // 64)
                    rbc = normp.tile([64, CH], F32, tag="rbc")
                    nc.gpsimd.dma_start(
                        out=rbc[:],
                        in_=bass.AP(rsrc.tensor, rsrc.offset + ridx * CH, [[0, 64], [1, CH]]))
                    if half == 0:
                        nc.vector.tensor_mul(
                            aT[0:64, pair, c * CH:(c + 1) * CH], oc[0:64, :], rbc[:])
                    else:
                        t64 = normp.tile([64, CH], BF16, tag="t64")
                        nc.vector.tensor_mul(t64[:], oc[0:64, :], rbc[:])
                        nc.gpsimd.dma_start(
                            out=aT[64:128, pair, c * CH:(c + 1) * CH], in_=t64[:])

            def normalize_pair_tail(c, pair, oc_lo, oc_hi, t_hi_out):
                # latency-critical final pair: reciprocal on the (now idle)
                # scalar engine, hi half to a scratch tile (no SBUF-bounce
                # DMA on the critical path; wo_tail contracts it at K=64).
                for half, oc in ((0, oc_lo), (64, oc_hi)):
                    rrow = normp.tile([1, CH], F32, tag="rrow")
                    nc.vector.reciprocal(rrow[:], oc[64:65, :])
                    bc = projps.tile([64, CH], F32, tag="ps")
                    nc.tensor.matmul(bc[:], lhsT=ones_t[:], rhs=rrow[:],
                                     start=True, stop=True)
                    if half == 0:
                        nc.vector.tensor_mul(
                            aT[0:64, pair, c * CH:(c + 1) * CH], oc[0:64, :], bc[:])
                    else:
                        nc.vector.tensor_mul(t_hi_out[:], oc[0:64, :], bc[:])

            def normalize_chunk(c, oc_tiles):
                if c == NCH - 1:
                    return  # handled per-pair inside attention_chunk
                for pair, oc_lo, oc_hi in oc_tiles:
                    normalize_pair_bounce(c, pair, oc_lo, oc_hi)

            def wo_stiles(c, which=(0, 1, 2, 3)):
                for stl in [4 * c + w for w in which]:
                    for n in range(2):
                        ps = projps.tile([128, CH], F32, tag="ps")
                        for pair in range(2):
                            nc.tensor.matmul(
                                ps[:], lhsT=aT[:, pair, stl * 128:(stl + 1) * 128],
                                rhs=wo_t[:, pair, n * CH:(n + 1) * CH],
                                start=(pair == 0), stop=(pair == 1),
                            )
                        ob = obufp.tile([128, CH], BF16, tag="ob")
                        nc.vector.tensor_copy(ob[:], ps[:])
                        nc.sync.dma_start(
                            out=out[:].rearrange("(t p) n -> p t n", p=128)[:, stl, n * CH:(n + 1) * CH],
                            in_=ob[:],
                        )

            def wo_tail(c, t_hi1):
                # last-chunk Wo: pair1-hi comes from the scratch tile at K=64
                # so no aT partition-shift DMA sits on the critical path.
                for stl in range(4 * c, 4 * c + 4):
                    for n in range(2):
                        ps = projps.tile([128, CH], F32, tag="ps")
                        lcol = (stl - 4 * c) * 128
                        nc.tensor.matmul(
                            ps[:], lhsT=aT[:, 0, stl * 128:(stl + 1) * 128],
                            rhs=wo_t[:, 0, n * CH:(n + 1) * CH],
                            start=True, stop=False,
                        )
                        nc.tensor.matmul(
                            ps[:], lhsT=aT[0:64, 1, stl * 128:(stl + 1) * 128],
                            rhs=wo_t[0:64, 1, n * CH:(n + 1) * CH],
                            start=False, stop=False,
                        )
                        nc.tensor.matmul(
                            ps[:], lhsT=t_hi1[:, lcol:lcol + 128],
                            rhs=wo_h[:, n * CH:(n + 1) * CH],
                            start=False, stop=True,
                        )
                        ob = obufp.tile([128, CH], BF16, tag="ob")
                        nc.scalar.copy(ob[:], ps[:])
                        nc.sync.dma_start(
                            out=out[:].rearrange("(t p) n -> p t n", p=128)[:, stl, n * CH:(n + 1) * CH],
                            in_=ob[:],
                        )

            with (
                tc.tile_pool(name="spair", bufs=2, space="PSUM") as spair,
                tc.tile_pool(name="opsum", bufs=2, space="PSUM") as opsum,
            ):
                ct_r = ct[:].rearrange("(mt p) s -> p mt s", p=128)
                st_r = st[:].rearrange("(mt p) s -> p mt s", p=128)
                # -- head: fire the critical input DMAs from three engines in
                # parallel so queue issue-rate doesn't serialize the start --
                pre = {}
                pre[(0, 'k')] = load_x_chunk(xk, 0, nc.sync, split=True)
                nc.gpsimd.dma_start(out=wk_t[:], in_=wk[:].rearrange("(kt p) e -> p kt e", p=128))
                nc.scalar.dma_start(out=ct_t[:, :, 0:CH], in_=ct_r[:, :, 0:CH])
                pre[(0, 'q')] = load_x_chunk(xq, 0, nc.scalar, split=True)
                nc.gpsimd.dma_start(out=st_t[:, :, 0:CH], in_=st_r[:, :, 0:CH])
                nc.gpsimd.dma_start(out=rt_t[:], in_=rt[:])
                nc.gpsimd.dma_start(out=wq_t[:], in_=wq[:].rearrange("(kt p) e -> p kt e", p=128))
                # PE warm-up on the memset tile (no DMA dependency): keeps the
                # HAM activity window busy while the inputs stream in.
                wps = spair.tile([128, 2 * CH], F32, tag="sp")
                for wi in range(16):
                    nc.tensor.matmul(
                        wps[:, 0:128], lhsT=warm_t[:], rhs=warm_t[:],
                        start=True, stop=True, skip_group_check=True,
                    )
                oc_by_chunk = {}
                for c in range(NCH):
                    with nc.named_scope(f"proj_c{c}"):
                        if c == 0:
                            nc.sync.dma_start(out=wv_t[:], in_=wv[:].rearrange("(kt p) e -> p kt e", p=128))
                            pre[(0, 'v')] = load_x_chunk(xv, 0, nc.sync)
                            nc.scalar.dma_start(out=tri_t[:], in_=tri[:])
                        proj_rope_chunk(pre[(c, 'k')], wk_t, kT, c)
                        proj_rope_chunk(pre[(c, 'q')], wq_t, qT, c)
                        if c == 0:
                            nc.gpsimd.dma_start(out=ct_t[:, :, CH:S], in_=ct_r[:, :, CH:S])
                            nc.gpsimd.dma_start(out=st_t[:, :, CH:S], in_=st_r[:, :, CH:S])
                            nc.gpsimd.dma_start(out=wo_t[:], in_=wo[:].rearrange("(pt p) n -> p pt n", p=128))
                            nc.gpsimd.dma_start(out=wo_h[:], in_=wo[:][192:256, :])
                        xt_v = pre[(c, 'v')]
                        # prefetch next chunk's activations ahead of the
                        # out-stores that attention/wo will enqueue
                        if c + 1 < NCH:
                            pre[(c + 1, 'k')] = load_x_chunk(xk, c + 1, nc.sync)
                            pre[(c + 1, 'q')] = load_x_chunk(xq, c + 1, nc.scalar)
                            pre[(c + 1, 'v')] = load_x_chunk(xv, c + 1, nc.sync)
                        for stl in range(4 * c, 4 * c + 4):
                            vproj_stile(xt_v, stl)
                    def filler(pair, c=c):
                        if c >= 1:
                            wo_stiles(c - 1, which=(2 * pair, 2 * pair + 1))
                    post_pair = None
                    if c == NCH - 1:
                        t_hi1 = normp.tile([64, CH], BF16, tag="t_hi1")

                        def post_pair(pair, oc_lo, oc_hi, c=c):
                            if pair == 0:
                                normalize_pair_bounce(c, pair, oc_lo, oc_hi)
                            else:
                                normalize_pair_tail(c, pair, oc_lo, oc_hi, t_hi1)
                    with nc.named_scope(f"att_c{c}"):
                        oc_by_chunk[c] = attention_chunk(c, spair, opsum, filler,
                                                         post_pair)
                    with nc.named_scope(f"norm_c{c}"):
                        normalize_chunk(c, oc_by_chunk[c])
                with nc.named_scope("wo_c3"):
                    wo_tail(NCH - 1, t_hi1)

    nc.compile()
    return nc


def _host_tables():
    inv_freq = 1.0 / (10000.0 ** (np.arange(0, DK, 2, dtype=np.float64) / DK))
    pos = np.arange(S, dtype=np.float64)
    fr = pos[:, None] * inv_freq[None, :]          # [S, 32]
    sc8 = 1.0 / math.sqrt(math.sqrt(DK))           # fold 1/sqrt(DK) as sqrt into q and k
    cosT = (np.cos(fr).T * sc8).astype(np.float32)  # [32, S]
    sinT = (np.sin(fr).T * sc8).astype(np.float32)
    C = np.zeros((E, S), np.float32)
    Sg = np.zeros((E, S), np.float32)
    for hh in range(4):
        C[hh * 64:hh * 64 + 32] = cosT
        C[hh * 64 + 32:hh * 64 + 64] = cosT
        Sg[hh * 64:hh * 64 + 32] = -sinT
        Sg[hh * 64 + 32:hh * 64 + 64] = sinT
    # half-swap permutation for two stacked heads (128 rows)
    R = np.zeros((128, 128), np.float32)
    for hh in range(2):
        for j in range(32):
            R[hh * 64 + j, hh * 64 + 32 + j] = 1.0
            R[hh * 64 + 32 + j, hh * 64 + j] = 1.0
    # TRIW[p, y] = 1 iff p <= y - 384; block with offset g uses cols [384-g, 896-g)
    y = np.arange(896)[None, :]
    p = np.arange(128)[:, None]
    TRI = (p <= y - 384).astype(np.float32)
    return C, Sg, R, TRI


_program_cache = {}


def kernel(q, k, v, mask, Wq, bq, Wk, bk, Wv, bv, Wo, bo):
    q = np.asarray(q, np.float32)
    k = np.asarray(k, np.float32)
    v = np.asarray(v, np.float32)
    mask = np.asarray(mask)
    Wq, bq = np.asarray(Wq, np.float32), np.asarray(bq, np.float32)
    Wk, bk = np.asarray(Wk, np.float32), np.asarray(bk, np.float32)
    Wv, bv = np.asarray(Wv, np.float32), np.asarray(bv, np.float32)
    Wo, bo = np.asarray(Wo, np.float32), np.asarray(bo, np.float32)

    causal = np.array_equal(
        np.asarray(mask[0, 0], np.int64), np.tril(np.ones((S, S), np.int64)))
    if not causal or np.any(bq) or np.any(bk):
        return _np_reference_fallback(q, k, v, mask, Wq, bq, Wk, bk, Wv, bv, Wo, bo)

    if "nc" not in _program_cache:
        _program_cache["nc"] = _build_program()
    nc = _program_cache["nc"]

    C, Sg, R, TRI = _host_tables()
    bf = ml_dtypes.bfloat16

    in_maps = []
    for c in range(8):
        b = c // 4
        h0 = 4 * (c % 4)
        sl = slice(h0 * DK, (h0 + 4) * DK)
        in_maps.append({
            "xq": np.ascontiguousarray(q[b].T).astype(bf),
            "xk": np.ascontiguousarray(k[b].T).astype(bf),
            "xv": np.ascontiguousarray(v[b].T).astype(bf),
            "wq": np.ascontiguousarray(Wq[sl].T).astype(bf),
            "wk": np.ascontiguousarray(Wk[sl].T).astype(bf),
            "wv": np.ascontiguousarray(Wv[sl].T).astype(bf),
            "wo": np.ascontiguousarray(Wo[:, sl].T).astype(bf),
            "ct": C.astype(bf),
            "st": Sg.astype(bf),
            "rt": R.astype(bf),
            "tri": TRI.astype(bf),
        })

    res = run_bass_kernel_spmd(nc, in_maps, core_ids=list(range(8)))

    out = np.zeros((B, S, D), np.float32)
    for c in range(8):
        out[c // 4] += res.results[c]["out"]
    # bv folds through softmax (rows sum to 1) and Wo; bo direct.
    out += (bv @ Wo.T + bo)[None, None, :]
    return out

